# revision 39
# baseline (speedup 1.0000x reference)
"""Trainium2 Bass kernel for one batched Kalman-filter update step.

Reference computation (jax):
    x_pred = F @ x                        # [64, 1]
    P_pred = F @ P @ F.T + Q              # [64, 64]
    y      = z - H @ x_pred               # [32, N]
    S      = H @ P_pred @ H.T + R         # [32, 32]
    K      = P_pred @ H.T @ inv(S)        # [64, 32]
    out    = x_pred + K @ y               # [64, N]

All the small-matrix work is O(64^3) and independent of N; the N-scaling
part collapses to `out[:, n] = K @ z[:, n] + c` with c = x_pred - K @ H @
x_pred.  That is a memory-bound streaming matmul over N = 1048576 columns.

Distribution: pure data parallel.  Each of the 8 NeuronCores handles
131072 columns of z; the tiny K/c are replicated.  No collectives are
needed (forward pass only).

Per-core device pipeline (Tile framework):
  - z shard viewed as blocks of [32, F_BLK]; each in-DMA brings 4 blocks
    into a [128, F_BLK] SBUF tile (partition p = 32*j + i holds z row i of
    block j).
  - Block-diagonal weights Wb [64, 128] = diag(K.T, K.T), stacked twice
    along partitions so both halves have a copy at their own base
    partition.  Matmuls take rhs = 64 partitions x 512 cols -> PSUM
    [128, 512]: rows 0-63 = K @ (block j), rows 64-127 = K @ (block j+1).
  - Vector/Scalar engines evacuate PSUM -> SBUF with a fused
    per-partition bias add (bias = [c; c]).
  - Out-DMAs write [128, F_BLK] SBUF tiles back as two [64, F_BLK] column
    blocks of the output shard.

DMA routing (measured on this silicon/runtime):
  - The gpsimd SWDGE ring sprays all 16 SDMA engines (~280 GB/s on pure
    HBM writes) but its HBM reads crawl (~6 GB/s/engine), so it carries
    outputs only.
  - HWDGE rings (sync/scalar) read at ~21-25 GB/s/engine but share a
    fixed pool of 4 SDMA engine slots (~85 GB/s total reads), so inputs
    ride them.
  - Issue-engine FIFOs are strict: a DMA whose wait isn't satisfied
    blocks everything behind it on that engine, so ScalarE (which also
    evacuates PSUM) only gets input tiles whose zin slot is guaranteed
    free at issue time.

Precision: COMPUTE_DTYPE="bf16" stages z/K in bf16 and writes the output
as bf16 (upcast to f32 on the host); accumulation stays fp32 in PSUM.
Relative error vs the f32 reference is ~2e-3 (gate is 2e-2).  Set
"f32" for full-precision staging (fp32r single-pass matmul, ~1.5e-4);
it is ~2x slower because this runtime's DMA read path caps at ~85 GB/s
per core, which doubles the time spent streaming the f32 input.
"""

import os

import numpy as np

import concourse.bass as bass
import concourse.mybir as mybir
from concourse import bacc
from concourse import tile
from concourse.bass_utils import run_bass_kernel_spmd

N_CORES = 8
STATE_DIM = 64
MEASURE_DIM = 32
N_TOTAL = 1048576
SHARD = N_TOTAL // N_CORES  # 131072 columns per core

COMPUTE_DTYPE = os.environ.get("AKF_DTYPE", "bf16")  # "bf16" | "f32"

MM_F = 512  # matmul moving free dim (one PSUM bank of fp32)


def _install_axon_ntff_hook():
    """Provide antenv.axon_hooks so run_bass_kernel_spmd(trace=True) can
    capture NTFF profiles under axon.  The agent image ships a stub antenv
    without axon_hooks; wire the ctypes-based hook from trn_agent_boot to
    the injected libaxon_pjrt.so.  Degrades to hook=None (tracing skipped,
    run still works) on any failure."""
    import sys
    import types

    if "antenv.axon_hooks" in sys.modules:
        return
    hook = None
    try:
        from trn_agent_boot.trn_boot import _ntff_profile_via_ctypes

        so_path = "/opt/axon/libaxon_pjrt.so"
        if os.path.exists(so_path):
            hook = _ntff_profile_via_ctypes(so_path)
    except Exception:
        hook = None
    mod = types.ModuleType("antenv.axon_hooks")
    state = {"hook": hook}
    mod.set_axon_ntff_profile_hook = lambda h: state.__setitem__("hook", h)
    mod.get_axon_ntff_profile_hook = lambda: state["hook"]
    sys.modules["antenv.axon_hooks"] = mod
    try:
        import antenv

        antenv.axon_hooks = mod
    except Exception:
        pass


_install_axon_ntff_hook()

_CACHE = {}


def _build_nc(shard: int, f_blk: int, dtype: str):
    """Build + compile the per-core Bass program (same program on all cores)."""
    n_tiles = shard // (4 * f_blk)
    n_slices = f_blk // MM_F
    f32 = mybir.dt.float32
    if dtype == "bf16":
        zdt = mybir.dt.bfloat16
        odt = mybir.dt.bfloat16
    else:
        # float32r end-to-end (same 4 bytes as f32 on the numpy side) so the
        # BIR verifier accepts the tiles as fp32r (single-pass) matmul inputs.
        zdt = mybir.dt.float32r
        odt = f32

    nc = bacc.Bacc("TRN2", target_bir_lowering=False, debug=False)

    z = nc.declare_dram_parameter("z", [MEASURE_DIM, shard], zdt, isOutput=False)
    w = nc.declare_dram_parameter("w", [128, 128], zdt, isOutput=False)
    cb = nc.declare_dram_parameter("c", [128, 1], f32, isOutput=False)
    out = nc.declare_dram_parameter("out", [STATE_DIM, shard], odt, isOutput=True)

    # Tile schedule: (f, col0) pairs, each covering z columns
    # [col0, col0 + 4*f).
    sched = []
    col = 0
    while col < shard:
        sched.append((f_blk, col))
        col += 4 * f_blk
    assert col == shard, (col, shard)

    zin_bufs = min(len(sched), 10)
    with tile.TileContext(nc) as tc:
        with (
            tc.tile_pool(name="const", bufs=1) as cpool,
            tc.tile_pool(name="zin", bufs=zin_bufs) as zpool,
            tc.tile_pool(name="zout", bufs=8) as opool,
            tc.tile_pool(name="ps", bufs=8, space="PSUM") as ppool,
        ):
            # Consts ride the scalar HWDGE ring: its small packets clear in
            # ~50 ns each, vs ~1 us receipt-bound packets on the SWDGE ring
            # (which would delay the first matmul by ~15 us).
            wt = cpool.tile([128, 128], zdt)
            nc.scalar.dma_start(out=wt[:, :], in_=w.ap()[:, :])
            ct = cpool.tile([128, 1], f32)
            nc.scalar.dma_start(out=ct[:, :], in_=cb.ap()[:, :])

            for t, (f, col0) in enumerate(sched):
                # [b, i, f] / [b, v, f] views at this tile's granularity
                Zf = z.ap().rearrange("i (b ff) -> b i ff", ff=f)
                Of = out.ap().rearrange("v (b ff) -> b v ff", ff=f)
                b0 = col0 // f
                zt = zpool.tile([128, f], zdt)
                in_eng = [nc.sync, nc.scalar, nc.gpsimd][t % 3]
                in_eng.dma_start(out=zt[:, :], in_=Zf[b0 : b0 + 4, :, :])
                for h in range(2):
                    ot = opool.tile([128, f], odt)
                    lhsT = wt[64 * h : 64 * h + 64, :]
                    for s in range(f // MM_F):
                        ps = ppool.tile([128, MM_F], f32)
                        rhs = zt[64 * h : 64 * h + 64, MM_F * s : MM_F * (s + 1)]
                        nc.tensor.matmul(ps[:, :], lhsT, rhs, start=True, stop=True)
                        if h == 0:
                            nc.vector.tensor_scalar_add(
                                ot[:, MM_F * s : MM_F * (s + 1)], ps[:, :], ct[:, :]
                            )
                        else:
                            nc.scalar.add(
                                ot[:, MM_F * s : MM_F * (s + 1)], ps[:, :], ct[:, :]
                            )
                    nc.gpsimd.dma_start(
                        out=Of[b0 + 2 * h : b0 + 2 * h + 2, :, :], in_=ot[:, :]
                    )

    nc.compile()
    return nc


def _f_blk(dtype: str) -> int:
    # 16 KB per-partition descriptor spans measured fastest on the HWDGE
    # read path for f32; for bf16 the smaller 8 KB descriptors win because
    # 8 finer tiles pipeline better than 4 coarse ones.
    return 4096


def _get_nc(dtype: str):
    key = (SHARD, _f_blk(dtype), dtype)
    if key not in _CACHE:
        _CACHE[key] = _build_nc(SHARD, _f_blk(dtype), dtype)
    return _CACHE[key]


def _host_params(F, H, Q, R, P, x):
    """The O(64^3) Kalman small-matrix chain, done once on the host."""
    F = np.asarray(F, dtype=np.float64)
    H = np.asarray(H, dtype=np.float64)
    Q = np.asarray(Q, dtype=np.float64)
    R = np.asarray(R, dtype=np.float64)
    P = np.asarray(P, dtype=np.float64)
    x = np.asarray(x, dtype=np.float64)

    x_pred = F @ x                       # [64, 1]
    P_pred = F @ P @ F.T + Q             # [64, 64]
    S = H @ P_pred @ H.T + R             # [32, 32]
    K = P_pred @ H.T @ np.linalg.inv(S)  # [64, 32]
    c = x_pred - K @ (H @ x_pred)        # [64, 1]

    K32 = K.astype(np.float32)
    # Block-diagonal [64, 128]: out rows 0-63 <- K @ rhs[0:32],
    # rows 64-127 <- K @ rhs[32:64]; stacked twice along partitions.
    Wb = np.zeros((64, 128), dtype=np.float32)
    Wb[0:32, 0:64] = K32.T
    Wb[32:64, 64:128] = K32.T
    W2 = np.concatenate([Wb, Wb], axis=0)          # [128, 128]
    c2 = np.concatenate([c, c], axis=0).astype(np.float32)  # [128, 1]
    return W2, c2


def _in_maps(z, W2, c2, dtype: str):
    if dtype == "bf16":
        import ml_dtypes

        zc = z.astype(ml_dtypes.bfloat16)
        W2 = W2.astype(ml_dtypes.bfloat16)
    else:
        zc = z
    return [
        {
            "z": np.ascontiguousarray(zc[:, i * SHARD : (i + 1) * SHARD]),
            "w": W2,
            "c": c2,
        }
        for i in range(N_CORES)
    ]


def kernel(z, F, H, Q, R, P, x):
    z = np.asarray(z, dtype=np.float32)
    assert z.shape == (MEASURE_DIM, N_TOTAL), z.shape
    W2, c2 = _host_params(F, H, Q, R, P, x)

    dtype = COMPUTE_DTYPE
    nc = _get_nc(dtype)
    in_maps = _in_maps(z, W2, c2, dtype)
    res = run_bass_kernel_spmd(nc, in_maps, core_ids=list(range(N_CORES)))
    out = np.concatenate([res.results[i]["out"] for i in range(N_CORES)], axis=1)
    if out.dtype != np.float32:
        out = out.astype(np.float32)
    return out


# revision 41
# speedup vs baseline: 1.0320x; 1.0320x over previous
"""Trainium2 Bass kernel for one batched Kalman-filter update step.

Reference computation (jax):
    x_pred = F @ x                        # [64, 1]
    P_pred = F @ P @ F.T + Q              # [64, 64]
    y      = z - H @ x_pred               # [32, N]
    S      = H @ P_pred @ H.T + R         # [32, 32]
    K      = P_pred @ H.T @ inv(S)        # [64, 32]
    out    = x_pred + K @ y               # [64, N]

All the small-matrix work is O(64^3) and independent of N; the N-scaling
part collapses to `out[:, n] = K @ z[:, n] + c` with c = x_pred - K @ H @
x_pred.  That is a memory-bound streaming matmul over N = 1048576 columns.

Distribution: pure data parallel.  Each of the 8 NeuronCores handles
131072 columns of z; the tiny K/c are replicated.  No collectives are
needed (forward pass only).

Per-core device pipeline (Tile framework):
  - z shard viewed as blocks of [32, F_BLK]; each in-DMA brings 4 blocks
    into a [128, F_BLK] SBUF tile (partition p = 32*j + i holds z row i of
    block j).
  - Block-diagonal weights Wb [64, 128] = diag(K.T, K.T), stacked twice
    along partitions so both halves have a copy at their own base
    partition.  Matmuls take rhs = 64 partitions x 512 cols -> PSUM
    [128, 512]: rows 0-63 = K @ (block j), rows 64-127 = K @ (block j+1).
  - Vector/Scalar engines evacuate PSUM -> SBUF with a fused
    per-partition bias add (bias = [c; c]).
  - Out-DMAs write [128, F_BLK] SBUF tiles back as two [64, F_BLK] column
    blocks of the output shard.

DMA routing (measured on this silicon/runtime):
  - The gpsimd SWDGE ring sprays all 16 SDMA engines (~280 GB/s on pure
    HBM writes) but its HBM reads crawl (~6 GB/s/engine), so it carries
    outputs only.
  - HWDGE rings (sync/scalar) read at ~21-25 GB/s/engine but share a
    fixed pool of 4 SDMA engine slots (~85 GB/s total reads), so inputs
    ride them.
  - Issue-engine FIFOs are strict: a DMA whose wait isn't satisfied
    blocks everything behind it on that engine, so ScalarE (which also
    evacuates PSUM) only gets input tiles whose zin slot is guaranteed
    free at issue time.

Precision: COMPUTE_DTYPE="bf16" stages z/K in bf16 and writes the output
as bf16 (upcast to f32 on the host); accumulation stays fp32 in PSUM.
Relative error vs the f32 reference is ~2e-3 (gate is 2e-2).  Set
"f32" for full-precision staging (fp32r single-pass matmul, ~1.5e-4);
it is ~2x slower because this runtime's DMA read path caps at ~85 GB/s
per core, which doubles the time spent streaming the f32 input.
"""

import os

import numpy as np

import concourse.bass as bass
import concourse.mybir as mybir
from concourse import bacc
from concourse import tile
from concourse.bass_utils import run_bass_kernel_spmd

N_CORES = 8
STATE_DIM = 64
MEASURE_DIM = 32
N_TOTAL = 1048576
SHARD = N_TOTAL // N_CORES  # 131072 columns per core

COMPUTE_DTYPE = os.environ.get("AKF_DTYPE", "bf16")  # "bf16" | "f32"

MM_F = 512  # matmul moving free dim (one PSUM bank of fp32)


def _install_axon_ntff_hook():
    """Provide antenv.axon_hooks so run_bass_kernel_spmd(trace=True) can
    capture NTFF profiles under axon.  The agent image ships a stub antenv
    without axon_hooks; wire the ctypes-based hook from trn_agent_boot to
    the injected libaxon_pjrt.so.  Degrades to hook=None (tracing skipped,
    run still works) on any failure."""
    import sys
    import types

    if "antenv.axon_hooks" in sys.modules:
        return
    hook = None
    try:
        from trn_agent_boot.trn_boot import _ntff_profile_via_ctypes

        so_path = "/opt/axon/libaxon_pjrt.so"
        if os.path.exists(so_path):
            hook = _ntff_profile_via_ctypes(so_path)
    except Exception:
        hook = None
    mod = types.ModuleType("antenv.axon_hooks")
    state = {"hook": hook}
    mod.set_axon_ntff_profile_hook = lambda h: state.__setitem__("hook", h)
    mod.get_axon_ntff_profile_hook = lambda: state["hook"]
    sys.modules["antenv.axon_hooks"] = mod
    try:
        import antenv

        antenv.axon_hooks = mod
    except Exception:
        pass


_install_axon_ntff_hook()

_CACHE = {}


def _build_nc(shard: int, f_blk: int, dtype: str):
    """Build + compile the per-core Bass program (same program on all cores)."""
    n_tiles = shard // (4 * f_blk)
    n_slices = f_blk // MM_F
    f32 = mybir.dt.float32
    if dtype == "bf16":
        zdt = mybir.dt.bfloat16
        odt = mybir.dt.bfloat16
    else:
        # float32r end-to-end (same 4 bytes as f32 on the numpy side) so the
        # BIR verifier accepts the tiles as fp32r (single-pass) matmul inputs.
        zdt = mybir.dt.float32r
        odt = f32

    nc = bacc.Bacc("TRN2", target_bir_lowering=False, debug=False)

    z = nc.declare_dram_parameter("z", [MEASURE_DIM, shard], zdt, isOutput=False)
    w = nc.declare_dram_parameter("w", [128, 128], zdt, isOutput=False)
    cb = nc.declare_dram_parameter("c", [128, 1], f32, isOutput=False)
    out = nc.declare_dram_parameter("out", [STATE_DIM, shard], odt, isOutput=True)

    # Tile schedule: (f, col0) pairs, each covering z columns
    # [col0, col0 + 4*f).
    sched = []
    col = 0
    while col < shard:
        sched.append((f_blk, col))
        col += 4 * f_blk
    assert col == shard, (col, shard)

    zin_bufs = min(len(sched), 10)
    with tile.TileContext(nc) as tc:
        with (
            tc.tile_pool(name="const", bufs=1) as cpool,
            tc.tile_pool(name="zin", bufs=zin_bufs) as zpool,
            tc.tile_pool(name="zout", bufs=8) as opool,
            tc.tile_pool(name="ps", bufs=8, space="PSUM") as ppool,
        ):
            # Consts ride the SWDGE ring ahead of any output traffic.
            wt = cpool.tile([128, 128], zdt)
            nc.gpsimd.dma_start(out=wt[:, :], in_=w.ap()[:, :])
            ct = cpool.tile([128, 1], f32)
            nc.gpsimd.dma_start(out=ct[:, :], in_=cb.ap()[:, :])

            for t, (f, col0) in enumerate(sched):
                # [b, i, f] / [b, v, f] views at this tile's granularity
                Zf = z.ap().rearrange("i (b ff) -> b i ff", ff=f)
                Of = out.ap().rearrange("v (b ff) -> b v ff", ff=f)
                b0 = col0 // f
                zt = zpool.tile([128, f], zdt)
                # ~1/4 of the reads ride the SWDGE ring to offload the 4
                # shared HWDGE engines, but only MID-stream tiles: their
                # issue sits behind earlier tiles' output DMAs in the
                # gpsimd FIFO, so their slow receipt-bound read packets
                # don't steal HWDGE engine cycles during the pipeline ramp.
                in_eng = [
                    nc.sync, nc.scalar, nc.sync, nc.scalar,
                    nc.gpsimd, nc.sync, nc.gpsimd, nc.scalar,
                ][t % 8]
                in_eng.dma_start(out=zt[:, :], in_=Zf[b0 : b0 + 4, :, :])
                for h in range(2):
                    ot = opool.tile([128, f], odt)
                    lhsT = wt[64 * h : 64 * h + 64, :]
                    for s in range(f // MM_F):
                        ps = ppool.tile([128, MM_F], f32)
                        rhs = zt[64 * h : 64 * h + 64, MM_F * s : MM_F * (s + 1)]
                        nc.tensor.matmul(ps[:, :], lhsT, rhs, start=True, stop=True)
                        if h == 0:
                            nc.vector.tensor_scalar_add(
                                ot[:, MM_F * s : MM_F * (s + 1)], ps[:, :], ct[:, :]
                            )
                        else:
                            nc.scalar.add(
                                ot[:, MM_F * s : MM_F * (s + 1)], ps[:, :], ct[:, :]
                            )
                    nc.gpsimd.dma_start(
                        out=Of[b0 + 2 * h : b0 + 2 * h + 2, :, :], in_=ot[:, :]
                    )

    nc.compile()
    return nc


def _f_blk(dtype: str) -> int:
    # 16 KB per-partition descriptor spans measured fastest on the HWDGE
    # read path for f32; for bf16 the smaller 8 KB descriptors win because
    # 8 finer tiles pipeline better than 4 coarse ones.
    return 4096


def _get_nc(dtype: str):
    key = (SHARD, _f_blk(dtype), dtype)
    if key not in _CACHE:
        _CACHE[key] = _build_nc(SHARD, _f_blk(dtype), dtype)
    return _CACHE[key]


def _host_params(F, H, Q, R, P, x):
    """The O(64^3) Kalman small-matrix chain, done once on the host."""
    F = np.asarray(F, dtype=np.float64)
    H = np.asarray(H, dtype=np.float64)
    Q = np.asarray(Q, dtype=np.float64)
    R = np.asarray(R, dtype=np.float64)
    P = np.asarray(P, dtype=np.float64)
    x = np.asarray(x, dtype=np.float64)

    x_pred = F @ x                       # [64, 1]
    P_pred = F @ P @ F.T + Q             # [64, 64]
    S = H @ P_pred @ H.T + R             # [32, 32]
    K = P_pred @ H.T @ np.linalg.inv(S)  # [64, 32]
    c = x_pred - K @ (H @ x_pred)        # [64, 1]

    K32 = K.astype(np.float32)
    # Block-diagonal [64, 128]: out rows 0-63 <- K @ rhs[0:32],
    # rows 64-127 <- K @ rhs[32:64]; stacked twice along partitions.
    Wb = np.zeros((64, 128), dtype=np.float32)
    Wb[0:32, 0:64] = K32.T
    Wb[32:64, 64:128] = K32.T
    W2 = np.concatenate([Wb, Wb], axis=0)          # [128, 128]
    c2 = np.concatenate([c, c], axis=0).astype(np.float32)  # [128, 1]
    return W2, c2


def _in_maps(z, W2, c2, dtype: str):
    if dtype == "bf16":
        import ml_dtypes

        zc = z.astype(ml_dtypes.bfloat16)
        W2 = W2.astype(ml_dtypes.bfloat16)
    else:
        zc = z
    return [
        {
            "z": np.ascontiguousarray(zc[:, i * SHARD : (i + 1) * SHARD]),
            "w": W2,
            "c": c2,
        }
        for i in range(N_CORES)
    ]


def kernel(z, F, H, Q, R, P, x):
    z = np.asarray(z, dtype=np.float32)
    assert z.shape == (MEASURE_DIM, N_TOTAL), z.shape
    W2, c2 = _host_params(F, H, Q, R, P, x)

    dtype = COMPUTE_DTYPE
    nc = _get_nc(dtype)
    in_maps = _in_maps(z, W2, c2, dtype)
    res = run_bass_kernel_spmd(nc, in_maps, core_ids=list(range(N_CORES)))
    out = np.concatenate([res.results[i]["out"] for i in range(N_CORES)], axis=1)
    if out.dtype != np.float32:
        out = out.astype(np.float32)
    return out
